# revision 1
# baseline (speedup 1.0000x reference)
"""GPT2 attention (B=2,S=2048,E=1024,H=16) on 8 NeuronCores.

Sharding: core c -> batch b=c//4, head-group g=c%4 (4 heads, d'=256 cols).
Per core (all matmuls in float32r: full PE rate with free dim >=256, fp32 data):
  - Q^T,K^T = (Wqk^T @ x)            [d,s] layout, bias per-partition (DVE)
  - V       = (x @ Wv_aug)           [s,d] layout, 65-col-per-head blocks with a
                                     ones column; bias added via broadcast tile
  - S^T     = K^T.T @ Q^T            [k,q] blocks (128k x 512q), causal block-skip
  - exp     on ACT over paired k-tile slabs [128, 1024]
  - mask    diagonal blocks: memset above-diag region + triangular mul (DVE)
  - O^T,Z   = V_aug.T @ expS^T       [65, 512] psum; row 64 = softmax denom Z
  - norm    recipZ (DVE) -> PE broadcast to 64 rows -> DVE mul -> attnT [256,2048]
  - partial c_proj = attnT.T @ Wp    [s, e], summed across head-groups on host
"""

import numpy as np

import concourse.bass as bass
import concourse.mybir as mybir
import concourse.tile as tile
from concourse import bacc
from concourse.bass_utils import run_bass_kernel_spmd

B, S, E, H = 2, 2048, 1024, 16
HD = 64           # head dim
HPC = 4           # heads per core
DP = HPC * HD     # 256 d' columns per core
NQC = 4           # q-chunks of 512
QCW = 512         # q-chunk width
NKT = S // 128    # 16 k-tiles
NST = S // 128    # 16 s-tiles
NET = E // 128    # 8 E-tiles

f32 = mybir.dt.float32
f32r = mybir.dt.float32r
bf16 = mybir.dt.bfloat16
FT = mybir.ActivationFunctionType

_CACHED = {}


def build_nc():
    nc = bacc.Bacc("TRN2", target_bir_lowering=False, debug=False,
                   enable_asserts=False, num_devices=8)

    xT = nc.dram_tensor("xT", [E, S], bf16, kind="ExternalInput")
    wqk = nc.dram_tensor("wqk", [E, 2 * DP], bf16, kind="ExternalInput")
    bqk = nc.dram_tensor("bqk", [128, 4], f32, kind="ExternalInput")
    wv = nc.dram_tensor("wv", [E, 260], bf16, kind="ExternalInput")
    vb = nc.dram_tensor("vb", [128, 260], f32, kind="ExternalInput")
    wp = nc.dram_tensor("wp", [DP, E], bf16, kind="ExternalInput")
    mask4 = nc.dram_tensor("mask4", [128, 2048], bf16, kind="ExternalInput")
    on = nc.dram_tensor("on", [1, 128], f32r, kind="ExternalInput")
    outp = nc.dram_tensor("outp", [S, E], f32, kind="ExternalOutput")

    with tile.TileContext(nc) as tc:
        with (
            nc.allow_low_precision("float32r is 4-byte fp32-layout data"),
            tc.tile_pool(name="consts", bufs=1) as consts,
            tc.tile_pool(name="acts", bufs=1) as acts,
            tc.tile_pool(name="slabs", bufs=5) as slabs,
            tc.tile_pool(name="small", bufs=3) as small,
            tc.tile_pool(name="outs", bufs=3) as outs,
            tc.tile_pool(name="ps", bufs=3, space="PSUM") as ps,
            tc.tile_pool(name="otps", bufs=2, space="PSUM") as otps,
        ):
            # ---- constants / weights in ----
            xt_sb = []
            wqk_sb = []
            wv_sb = []
            for t in range(NET):
                xt = consts.tile([128, S], bf16, tag=f"xt{t}")
                nc.sync.dma_start(xt[:], xT[t * 128:(t + 1) * 128, :])
                xt_sb.append(xt)
                wq = consts.tile([128, 2 * DP], bf16, tag=f"wqk{t}")
                nc.sync.dma_start(wq[:], wqk[t * 128:(t + 1) * 128, :])
                wqk_sb.append(wq)
                wvt = consts.tile([128, 260], bf16, tag=f"wv{t}")
                nc.sync.dma_start(wvt[:], wv[t * 128:(t + 1) * 128, :])
                wv_sb.append(wvt)
            vb_sb = consts.tile([128, 260], f32, tag="vb")
            nc.sync.dma_start(vb_sb[:], vb[:, :])
            bqk_sb = consts.tile([128, 4], f32, tag="bqk")
            nc.sync.dma_start(bqk_sb[:], bqk[:, :])
            wp_sb = []
            for t in range(2):
                wpt = consts.tile([128, E], bf16, tag=f"wp{t}")
                nc.sync.dma_start(wpt[:], wp[t * 128:(t + 1) * 128, :])
                wp_sb.append(wpt)
            mask4_sb = consts.tile([128, 2048], bf16, tag="mask4")
            nc.sync.dma_start(mask4_sb[:], mask4[:, :])
            on_sb = consts.tile([1, 128], f32r, tag="on")
            nc.sync.dma_start(on_sb[:], on[:, :])

            # ---- persistent activations ----
            v_sb = [acts.tile([128, 260], bf16, tag=f"v{st}", name=f"v{st}") for st in range(NST)]
            qkt_sb = [acts.tile([128, S], bf16, tag=f"qkt{t}", name=f"qkt{t}") for t in range(4)]
            attnT_sb = [acts.tile([128, S], bf16, tag=f"attnT{t}", name=f"attnT{t}") for t in range(2)]

            # ---- V projection: V_aug[s, 260] = x @ Wv_aug + vb ----
            for st in range(NST):
                vps = ps.tile([128, 260], f32, tag="ps")
                for kt in range(NET):
                    nc.tensor.matmul(
                        vps[:],
                        (xt_sb[kt][:, st * 128:(st + 1) * 128]),
                        (wv_sb[kt][:]),
                        start=(kt == 0), stop=(kt == NET - 1),
                    )
                nc.vector.tensor_add(v_sb[st][:], vps[:], vb_sb[:])

            # ---- QK^T projection: [d, s] = Wqk.T @ x (+bias per-partition) ----
            for t in (0, 2, 1, 3):
                for sc in range(4):
                    qps = ps.tile([128, 512], f32, tag="ps")
                    for kt in range(NET):
                        nc.tensor.matmul(
                            qps[:],
                            (wqk_sb[kt][:, t * 128:(t + 1) * 128]),
                            (xt_sb[kt][:, sc * 512:(sc + 1) * 512]),
                            start=(kt == 0), stop=(kt == NET - 1),
                        )
                    nc.vector.tensor_scalar_add(
                        qkt_sb[t][:, sc * 512:(sc + 1) * 512],
                        qps[:], bqk_sb[:, t:t + 1])

            # ---- attention: head pairs interleaved at k-pair level ----
            for qc in range(NQC):
                nkt = 4 * qc + 4  # causal: k-tiles 0 .. 4qc+3
                for hp in range(2):
                    ots = [otps.tile([65, 512], f32, tag="ot",
                                     name=f"ot{qc}_{hp}_{i}") for i in range(2)]
                    for kp in range(nkt // 2):  # k-tile pairs
                        for i in range(2):
                            h = 2 * hp + i
                            tq = h // 2
                            po = (h % 2) * 64
                            qt_ap = qkt_sb[tq]
                            kt_ap = qkt_sb[2 + tq]
                            sp = ps.tile([128, 1024], f32, tag="ps")
                            for half in range(2):
                                kt = 2 * kp + half
                                nc.tensor.matmul(
                                    sp[:, half * 512:(half + 1) * 512],
                                    (kt_ap[po:po + 64, kt * 128:(kt + 1) * 128]),
                                    (qt_ap[po:po + 64, qc * 512:(qc + 1) * 512]),
                                    start=True, stop=True,
                                )
                            slab = slabs.tile([128, 1024], bf16, tag="slab")
                            nc.scalar.activation(slab[:], sp[:], FT.Exp)
                            for half in range(2):
                                kt = 2 * kp + half
                                di = kt - 4 * qc  # diagonal sub-position
                                if di >= 0:
                                    base = half * 512
                                    nc.vector.tensor_mul(
                                        slab[:, base:base + 512],
                                        slab[:, base:base + 512],
                                        mask4_sb[:, di * 512:(di + 1) * 512])
                            for half in range(2):
                                kt = 2 * kp + half
                                nc.tensor.matmul(
                                    ots[i][:],
                                    (v_sb[kt][:, 65 * h:65 * h + 65]),
                                    (slab[:, half * 512:(half + 1) * 512]),
                                    start=(kt == 0), stop=(kt == nkt - 1),
                                )
                    # normalize: rows 0..63 * (1/Z), Z = row 64
                    for i in range(2):
                        h = 2 * hp + i
                        po = (h % 2) * 64
                        zrow = small.tile([1, 512], f32, tag="zrow")
                        nc.vector.tensor_copy(zrow[:], ots[i][64:65, :])
                        rz = small.tile([1, 512], f32, tag="rz")
                        nc.vector.reciprocal_approx_fast(rz[:], zrow[:])
                        sbb = small.tile([64, 512], f32, tag="sbb")
                        nc.gpsimd.partition_broadcast(sbb[:], rz[0:1, :])
                        nc.vector.tensor_mul(
                            attnT_sb[h // 2][po:po + 64,
                                             qc * 512:(qc + 1) * 512],
                            ots[i][0:64, :], sbb[:])

                # ---- c_proj for the 4 s-tiles of this q-chunk ----
                for sti in range(4):
                    st = 4 * qc + sti
                    for nchk in range(2):
                        cps = ps.tile([128, 512], f32, tag="ps")
                        for kt2 in range(2):
                            nc.tensor.matmul(
                                cps[:],
                                (attnT_sb[kt2][:, st * 128:(st + 1) * 128]),
                                (wp_sb[kt2][:, nchk * 512:(nchk + 1) * 512]),
                                start=(kt2 == 0), stop=(kt2 == 1),
                            )
                        ob = outs.tile([128, 512], f32, tag="ob")
                        nc.vector.tensor_copy(ob[:], cps[:])
                        nc.sync.dma_start(
                            outp[st * 128:(st + 1) * 128,
                                 nchk * 512:(nchk + 1) * 512], ob[:])

    nc.finalize()
    return nc


def _prep_inputs(hidden_states, w_attn, b_attn, w_proj, b_proj):
    hs = np.asarray(hidden_states, np.float32)
    wa = np.asarray(w_attn, np.float32)
    ba = np.asarray(b_attn, np.float32)
    wpj = np.asarray(w_proj, np.float32)

    import ml_dtypes
    bfl = ml_dtypes.bfloat16
    xTs = [np.ascontiguousarray(hs[b].T.astype(bfl)) for b in range(B)]
    triu = (np.arange(128)[:, None] <= np.arange(128)[None, :]).astype(np.float32)
    mask4 = np.zeros((128, 2048), np.float32)
    for i in range(4):
        m = np.ones((128, 512), np.float32)
        m[:, :i * 128] = 0.0
        m[:, i * 128:(i + 1) * 128] = triu
        mask4[:, i * 512:(i + 1) * 512] = m
    mask4 = mask4.astype(ml_dtypes.bfloat16)
    on = np.ones((1, 128), np.float32)

    in_maps = []
    for c in range(8):
        b, g = c // 4, c % 4
        q0 = DP * g
        k0 = E + DP * g
        v0 = 2 * E + DP * g
        wqk = np.concatenate(
            [wa[:, q0:q0 + DP] * 0.125, wa[:, k0:k0 + DP]], axis=1).astype(bfl)
        bqk = np.zeros((128, 4), np.float32)
        bqk[:, 0] = ba[q0:q0 + 128] * 0.125
        bqk[:, 1] = ba[q0 + 128:q0 + 256] * 0.125
        bqk[:, 2] = ba[k0:k0 + 128]
        bqk[:, 3] = ba[k0 + 128:k0 + 256]
        wv = np.zeros((E, 260), bfl)
        vb = np.zeros((128, 260), np.float32)
        for h in range(HPC):
            wv[:, 65 * h:65 * h + 64] = wa[:, v0 + 64 * h:v0 + 64 * h + 64].astype(bfl)
            vb[:, 65 * h:65 * h + 64] = ba[v0 + 64 * h:v0 + 64 * h + 64]
            vb[:, 65 * h + 64] = 1.0
        wp = np.ascontiguousarray(wpj[DP * g:DP * (g + 1), :].astype(bfl))
        in_maps.append({
            "xT": xTs[b],
            "wqk": np.ascontiguousarray(wqk),
            "bqk": bqk,
            "wv": wv,
            "vb": vb,
            "wp": wp,
            "mask4": mask4,
            "on": on,
        })
    return in_maps


def run(trace=False, **inputs):
    if "nc" not in _CACHED:
        _CACHED["nc"] = build_nc()
    nc = _CACHED["nc"]
    in_maps = _prep_inputs(**inputs)
    res = run_bass_kernel_spmd(nc, in_maps, list(range(8)), trace=trace)
    b_proj = np.asarray(inputs["b_proj"], np.float32)
    out = np.empty((B, S, E), np.float32)
    for b in range(B):
        acc = res.results[4 * b]["outp"].astype(np.float32)
        for g in range(1, 4):
            acc = acc + res.results[4 * b + g]["outp"]
        out[b] = acc + b_proj
    return out, res


def kernel(**inputs):
    out, _ = run(trace=False, **inputs)
    return out



# revision 6
# speedup vs baseline: 1.3511x; 1.3511x over previous
"""GPT2 attention (B=2,S=2048,E=1024,H=16) on 8 NeuronCores.

Sharding: core c -> batch b=c//4, head-group g=c%4 (4 heads, d'=256 cols).

Per core (all matmul data bf16):
  - Q^T,K^T = (Wqk^T @ x)      [d, s] layout; bias per-partition (DVE)
  - V       = (x @ Wv_aug)     [s, 260]; 65-col blocks per head, ones col via bias
  - S^T     = K^T.T @ Q^T      [k, q]; TWO heads packed per PE pass via 64-row
                               tile_position (0,0)/(64,0) into one 2-bank psum
                               slab [128, 1024]; causal trapezoid column skip
  - exp     one ACT pass per slab (3D AP over both head halves)
  - mask    diagonal 128-blocks: triangular mul (DVE)
  - O^T,Z   = V_aug.T @ expS^T [65, 512] psum; row 64 = softmax denom Z
  - norm    recipZ (DVE) -> gpsimd partition broadcast -> DVE mul -> attnT
  - c_proj  attnT.T @ Wp, psum pair [128,1024] -> bf16 -> DMA; partial sums
            over head-groups reduced on host.
Emission interleaves projection pair-groups and c_proj(qc-1) into the
attention stream so the PE never drains while ACT works through the exps.
"""

import numpy as np

import concourse.bass as bass
import concourse.mybir as mybir
import concourse.tile as tile
from concourse import bacc
from concourse.bass_utils import run_bass_kernel_spmd

B, S, E, H = 2, 2048, 1024, 16
HD = 64           # head dim
HPC = 4           # heads per core
DP = HPC * HD     # 256 d' columns per core
NQC = 4           # q-chunks of 512
NET = E // 128    # 8 E-tiles
NST = S // 128    # 16 s-tiles

f32 = mybir.dt.float32
bf16 = mybir.dt.bfloat16
FT = mybir.ActivationFunctionType

_CACHED = {}


def build_nc():
    nc = bacc.Bacc("TRN2", target_bir_lowering=False, debug=False,
                   enable_asserts=False, num_devices=8)

    xT = nc.dram_tensor("xT", [E, S], bf16, kind="ExternalInput")
    wqk = nc.dram_tensor("wqk", [E, 2 * DP], bf16, kind="ExternalInput")
    bqk = nc.dram_tensor("bqk", [128, 4], f32, kind="ExternalInput")
    wv = nc.dram_tensor("wv", [E, 260], bf16, kind="ExternalInput")
    vb = nc.dram_tensor("vb", [128, 260], f32, kind="ExternalInput")
    wp = nc.dram_tensor("wp", [DP, E], bf16, kind="ExternalInput")
    trimask = nc.dram_tensor("trimask", [128, 128], bf16, kind="ExternalInput")
    outp = nc.dram_tensor("outp", [S, E], bf16, kind="ExternalOutput")

    with tile.TileContext(nc) as tc:
        with (
            tc.tile_pool(name="consts", bufs=1) as consts,
            tc.tile_pool(name="acts", bufs=1) as acts,
            tc.tile_pool(name="slabs", bufs=4) as slabs,
            tc.tile_pool(name="small", bufs=2) as small,
            tc.tile_pool(name="outs", bufs=2) as outs,
            tc.tile_pool(name="ps", bufs=1, space="PSUM") as ps,
        ):
            # ---- constant / weight DMAs (sync queue, deadline order) ----
            bqk_sb = consts.tile([128, 4], f32, tag="bqk")
            nc.sync.dma_start(bqk_sb[:], bqk[:, :])
            vb_sb = consts.tile([128, 260], f32, tag="vb")
            nc.sync.dma_start(vb_sb[:], vb[:, :])
            tri_sb = consts.tile([128, 128], bf16, tag="tri")
            nc.sync.dma_start(tri_sb[:], trimask[:, :])

            # warm up the ACT exp table early (table load ~2.7us)
            warm = consts.tile([128, 4], f32, tag="warm")
            nc.scalar.activation(warm[:], bqk_sb[:], FT.Exp)

            xt_sb = []
            wqk_sb = []
            wv_sb = []
            for t in range(NET):
                wq = consts.tile([128, 2 * DP], bf16, tag=f"wqk{t}")
                nc.sync.dma_start(wq[:], wqk[t * 128:(t + 1) * 128, :])
                wqk_sb.append(wq)
                wvt = consts.tile([128, 260], bf16, tag=f"wv{t}")
                nc.sync.dma_start(wvt[:], wv[t * 128:(t + 1) * 128, :])
                wv_sb.append(wvt)
                xt = consts.tile([128, S], bf16, tag=f"xt{t}")
                nc.sync.dma_start(xt[:], xT[t * 128:(t + 1) * 128, :])
                xt_sb.append(xt)
            wp_sb = []
            for t in range(2):
                wpt = consts.tile([128, E], bf16, tag=f"wp{t}")
                nc.sync.dma_start(wpt[:], wp[t * 128:(t + 1) * 128, :])
                wp_sb.append(wpt)

            # ---- persistent activations ----
            v_sb = [acts.tile([128, 260], bf16, tag=f"v{st}", name=f"v{st}")
                    for st in range(NST)]
            qkt_sb = [acts.tile([128, S], bf16, tag=f"qkt{t}", name=f"qkt{t}")
                      for t in range(4)]
            attnT_sb = [acts.tile([128, S], bf16, tag=f"attnT{t}",
                                  name=f"attnT{t}") for t in range(2)]

            # ---- projection pair-group emitters ----
            def qk_pair(tA, scA, tB, scB, tag):
                """two QK projection column groups through one 2-bank psum."""
                p = ps.tile([128, 1024], f32, tag=tag, bufs=PS_BUFS[tag],
                            name=f"qkp{tA}{scA}{tB}{scB}")
                for kt in range(NET):
                    nc.tensor.matmul(
                        p[:, 0:512],
                        (wqk_sb[kt][:, tA * 128:(tA + 1) * 128]),
                        (xt_sb[kt][:, scA * 512:(scA + 1) * 512]),
                        start=(kt == 0), stop=(kt == NET - 1))
                    nc.tensor.matmul(
                        p[:, 512:1024],
                        (wqk_sb[kt][:, tB * 128:(tB + 1) * 128]),
                        (xt_sb[kt][:, scB * 512:(scB + 1) * 512]),
                        start=(kt == 0), stop=(kt == NET - 1))
                nc.vector.tensor_scalar_add(
                    qkt_sb[tA][:, scA * 512:(scA + 1) * 512],
                    p[:, 0:512], bqk_sb[:, tA:tA + 1])
                nc.vector.tensor_scalar_add(
                    qkt_sb[tB][:, scB * 512:(scB + 1) * 512],
                    p[:, 512:1024], bqk_sb[:, tB:tB + 1])

            def v_pair(st0, st1, tag):
                p = ps.tile([128, 1024], f32, tag=tag, bufs=PS_BUFS[tag],
                            name=f"vp{st0}")
                for kt in range(NET):
                    nc.tensor.matmul(
                        p[:, 0:260],
                        (xt_sb[kt][:, st0 * 128:(st0 + 1) * 128]),
                        (wv_sb[kt][:]),
                        start=(kt == 0), stop=(kt == NET - 1))
                    nc.tensor.matmul(
                        p[:, 512:772],
                        (xt_sb[kt][:, st1 * 128:(st1 + 1) * 128]),
                        (wv_sb[kt][:]),
                        start=(kt == 0), stop=(kt == NET - 1))
                nc.vector.tensor_add(v_sb[st0][:], p[:, 0:260], vb_sb[:])
                nc.vector.tensor_add(v_sb[st1][:], p[:, 512:772], vb_sb[:])

            def v_single(st, tag):
                p = ps.tile([128, 512], f32, tag=tag, bufs=PS_BUFS[tag],
                            name=f"vs{st}")
                for kt in range(NET):
                    nc.tensor.matmul(
                        p[:, 0:260],
                        (xt_sb[kt][:, st * 128:(st + 1) * 128]),
                        (wv_sb[kt][:]),
                        start=(kt == 0), stop=(kt == NET - 1))
                nc.vector.tensor_add(v_sb[st][:], p[:, 0:260], vb_sb[:])

            def cproj_pair(st):
                """c_proj for s-tile st, both 512-col output chunks."""
                p = ps.tile([128, 1024], f32, tag="pj", bufs=PS_BUFS["pj"],
                            name=f"cp{st}")
                for nchk in range(2):
                    for i2 in range(2):
                        nc.tensor.matmul(
                            p[:, nchk * 512:(nchk + 1) * 512],
                            (attnT_sb[i2][:, st * 128:(st + 1) * 128]),
                            (wp_sb[i2][:, nchk * 512:(nchk + 1) * 512]),
                            start=(i2 == 0), stop=(i2 == 1))
                ob = outs.tile([128, 1024], bf16, tag="ob")
                nc.vector.tensor_copy(ob[:], p[:])
                nc.sync.dma_start(outp[st * 128:(st + 1) * 128, :], ob[:])

            PS_BUFS = {"sp": 2, "ot": 2, "pj": 1}

            # ---- filler schedule: (qc, hp, round) -> emitters ----
            fillers = {}

            def place(key, fn):
                fillers.setdefault(key, []).append(fn)

            # pre-attention wave (uses sp/ot slots before attention claims them)
            pre = [
                lambda: qk_pair(0, 0, 2, 0, "sp"),
                lambda: v_pair(0, 1, "sp"),
                lambda: v_single(2, "ot"),
                lambda: v_single(3, "ot"),
            ]
            place((0, 0, 0), lambda: qk_pair(1, 0, 3, 0, "pj"))
            place((0, 0, 1), lambda: qk_pair(0, 1, 1, 1, "pj"))   # QQ sc1
            place((0, 1, 0), lambda: v_pair(4, 5, "pj"))
            place((0, 1, 1), lambda: v_pair(6, 7, "pj"))
            place((1, 0, 0), lambda: qk_pair(2, 1, 3, 1, "pj"))   # KK sc1
            place((1, 0, 1), lambda: qk_pair(0, 2, 1, 2, "pj"))   # QQ sc2
            place((1, 0, 2), lambda: v_pair(8, 9, "pj"))
            place((1, 1, 0), lambda: v_pair(10, 11, "pj"))
            place((1, 1, 1), lambda: cproj_pair(0))
            place((1, 1, 2), lambda: cproj_pair(1))
            place((1, 1, 3), lambda: cproj_pair(2))
            place((2, 0, 0), lambda: cproj_pair(3))
            place((2, 0, 1), lambda: qk_pair(2, 2, 3, 2, "pj"))   # KK sc2
            place((2, 0, 2), lambda: qk_pair(0, 3, 1, 3, "pj"))   # QQ sc3
            place((2, 0, 4), lambda: v_pair(12, 13, "pj"))
            place((2, 1, 0), lambda: v_pair(14, 15, "pj"))
            place((2, 1, 2), lambda: cproj_pair(4))
            place((2, 1, 3), lambda: cproj_pair(5))
            place((2, 1, 4), lambda: cproj_pair(6))
            place((3, 0, 0), lambda: cproj_pair(7))
            place((3, 0, 1), lambda: qk_pair(2, 3, 3, 3, "pj"))   # KK sc3
            place((3, 0, 3), lambda: cproj_pair(8))
            place((3, 0, 5), lambda: cproj_pair(9))
            place((3, 1, 1), lambda: cproj_pair(10))
            place((3, 1, 3), lambda: cproj_pair(11))
            # qc3's own c_proj (st 12-15) must follow qc3's normalize -> tail
            tail = [lambda st=st: cproj_pair(st) for st in range(12, 16)]

            for fn in pre:
                fn()

            # ---- attention ----
            for qc in range(NQC):
                nkt = 4 * qc + 4
                for hp in range(2):
                    tq, tk = hp, 2 + hp
                    ots = [ps.tile([65, 512], f32, tag="ot",
                                   bufs=PS_BUFS["ot"],
                                   name=f"ot{qc}_{hp}_{i}") for i in range(2)]
                    for r in range(nkt // 2):
                        for fn in fillers.pop((qc, hp, r), []):
                            fn()
                        slabs_r = []
                        for kt in (2 * r, 2 * r + 1):
                            di = kt - 4 * qc
                            o = max(di, 0) * 128
                            sp = ps.tile([128, 1024], f32, tag="sp",
                                         bufs=PS_BUFS["sp"], name="sp")
                            for i in range(2):
                                nc.tensor.matmul(
                                    sp[:, 512 * i + o:512 * i + 512],
                                    (qkt_sb[tk][64 * i:64 * i + 64,
                                                kt * 128:(kt + 1) * 128]),
                                    (qkt_sb[tq][64 * i:64 * i + 64,
                                                qc * 512 + o:(qc + 1) * 512]),
                                    start=True, stop=True)
                            slab = slabs.tile([128, 1024], bf16, tag="slab")
                            if o == 0:
                                nc.scalar.activation(slab[:], sp[:], FT.Exp)
                            elif o >= 256:
                                # two small exps cheaper than one full slab
                                for i in range(2):
                                    nc.scalar.activation(
                                        slab[:, 512 * i + o:512 * i + 512],
                                        sp[:, 512 * i + o:512 * i + 512],
                                        FT.Exp)
                            else:
                                # full-width exp; cols < o are never read
                                nc.scalar.activation(slab[:], sp[:], FT.Exp)
                            if di >= 0:
                                for i in range(2):
                                    nc.vector.tensor_mul(
                                        slab[:, 512 * i + o:512 * i + o + 128],
                                        slab[:, 512 * i + o:512 * i + o + 128],
                                        tri_sb[:])
                            slabs_r.append((kt, o, slab))
                        for kt, o, slab in slabs_r:
                            for i in range(2):
                                h = 2 * hp + i
                                nc.tensor.matmul(
                                    ots[i][:, o:512],
                                    (v_sb[kt][:, 65 * h:65 * h + 65]),
                                    (slab[:, 512 * i + o:512 * i + 512]),
                                    start=(kt == 0), stop=(kt == nkt - 1))
                    # normalize: rows 0..63 * (1/Z), Z = row 64
                    for i in range(2):
                        h = 2 * hp + i
                        zrow = small.tile([1, 512], f32, tag="zrow")
                        nc.vector.tensor_copy(zrow[:], ots[i][64:65, :])
                        rz = small.tile([1, 512], f32, tag="rz")
                        nc.vector.reciprocal_approx_fast(rz[:], zrow[:])
                        sbb = small.tile([64, 512], f32, tag="sbb")
                        nc.gpsimd.partition_broadcast(sbb[:], rz[0:1, :])
                        nc.vector.tensor_mul(
                            attnT_sb[hp][64 * i:64 * i + 64,
                                         qc * 512:(qc + 1) * 512],
                            ots[i][0:64, :], sbb[:])
            for key in sorted(fillers):
                for fn in fillers[key]:
                    fn()
            for fn in tail:
                fn()

    nc.finalize()
    return nc


def _prep_inputs(hidden_states, w_attn, b_attn, w_proj, b_proj):
    hs = np.asarray(hidden_states, np.float32)
    wa = np.asarray(w_attn, np.float32)
    ba = np.asarray(b_attn, np.float32)
    wpj = np.asarray(w_proj, np.float32)

    import ml_dtypes
    bfl = ml_dtypes.bfloat16
    xTs = [np.ascontiguousarray(hs[b].T.astype(bfl)) for b in range(B)]
    triu = (np.arange(128)[:, None] <= np.arange(128)[None, :])
    trimask = triu.astype(bfl)

    in_maps = []
    for c in range(8):
        b, g = c // 4, c % 4
        q0 = DP * g
        k0 = E + DP * g
        v0 = 2 * E + DP * g
        wqk = np.concatenate(
            [wa[:, q0:q0 + DP] * 0.125, wa[:, k0:k0 + DP]], axis=1).astype(bfl)
        bqk = np.zeros((128, 4), np.float32)
        bqk[:, 0] = ba[q0:q0 + 128] * 0.125
        bqk[:, 1] = ba[q0 + 128:q0 + 256] * 0.125
        bqk[:, 2] = ba[k0:k0 + 128]
        bqk[:, 3] = ba[k0 + 128:k0 + 256]
        wv = np.zeros((E, 260), bfl)
        vb = np.zeros((128, 260), np.float32)
        for h in range(HPC):
            wv[:, 65 * h:65 * h + 64] = wa[:, v0 + 64 * h:v0 + 64 * h + 64].astype(bfl)
            vb[:, 65 * h:65 * h + 64] = ba[v0 + 64 * h:v0 + 64 * h + 64]
            vb[:, 65 * h + 64] = 1.0
        wp = np.ascontiguousarray(wpj[DP * g:DP * (g + 1), :].astype(bfl))
        in_maps.append({
            "xT": xTs[b],
            "wqk": np.ascontiguousarray(wqk),
            "bqk": bqk,
            "wv": wv,
            "vb": vb,
            "wp": wp,
            "trimask": trimask,
        })
    return in_maps


def run(trace=False, **inputs):
    if "nc" not in _CACHED:
        _CACHED["nc"] = build_nc()
    nc = _CACHED["nc"]
    in_maps = _prep_inputs(**inputs)
    res = run_bass_kernel_spmd(nc, in_maps, list(range(8)), trace=trace)
    b_proj = np.asarray(inputs["b_proj"], np.float32)
    out = np.empty((B, S, E), np.float32)
    for b in range(B):
        acc = res.results[4 * b]["outp"].astype(np.float32)
        for g in range(1, 4):
            acc = acc + res.results[4 * b + g]["outp"].astype(np.float32)
        out[b] = acc + b_proj
    return out, res


def kernel(**inputs):
    out, _ = run(trace=False, **inputs)
    return out


# revision 10
# speedup vs baseline: 1.3856x; 1.0256x over previous
"""GPT2 attention (B=2,S=2048,E=1024,H=16) on 8 NeuronCores.

Sharding: core c -> batch b=c//4, head-group g=c%4 (4 heads, d'=256 cols).

Per core (all matmul data bf16):
  - Q^T,K^T = (Wqk^T @ x)      [d, s] layout; bias per-partition (DVE)
  - V       = (x @ Wv_aug)     [s, 260]; 65-col blocks per head, ones col via bias
  - S^T     = K^T.T @ Q^T      [k, q]; TWO heads packed per PE pass via 64-row
                               tile_position (0,0)/(64,0) into one 2-bank psum
                               slab [128, 1024]; causal trapezoid column skip
  - exp     one ACT pass per slab (3D AP over both head halves)
  - mask    diagonal 128-blocks: triangular mul (DVE)
  - O^T,Z   = V_aug.T @ expS^T [65, 512] psum; row 64 = softmax denom Z
  - norm    recipZ (DVE) -> gpsimd partition broadcast -> DVE mul -> attnT
  - c_proj  attnT.T @ Wp, psum pair [128,1024] -> bf16 -> DMA; partial sums
            over head-groups reduced on host.
Emission interleaves projection pair-groups and c_proj(qc-1) into the
attention stream so the PE never drains while ACT works through the exps.
"""

import numpy as np

import concourse.bass as bass
import concourse.mybir as mybir
import concourse.tile as tile
from concourse import bacc
from concourse.bass_utils import run_bass_kernel_spmd

B, S, E, H = 2, 2048, 1024, 16
HD = 64           # head dim
HPC = 4           # heads per core
DP = HPC * HD     # 256 d' columns per core
NQC = 4           # q-chunks of 512
NET = E // 128    # 8 E-tiles
NST = S // 128    # 16 s-tiles

f32 = mybir.dt.float32
bf16 = mybir.dt.bfloat16
FT = mybir.ActivationFunctionType

_CACHED = {}


def build_nc():
    nc = bacc.Bacc("TRN2", target_bir_lowering=False, debug=False,
                   enable_asserts=False, num_devices=8)

    xT = nc.dram_tensor("xT", [E, S], bf16, kind="ExternalInput")
    wqk = nc.dram_tensor("wqk", [E, 2 * DP], bf16, kind="ExternalInput")
    bqk = nc.dram_tensor("bqk", [128, 4], f32, kind="ExternalInput")
    wv = nc.dram_tensor("wv", [E, 260], bf16, kind="ExternalInput")
    vb = nc.dram_tensor("vb", [128, 260], f32, kind="ExternalInput")
    wp = nc.dram_tensor("wp", [DP, E], bf16, kind="ExternalInput")
    trimask = nc.dram_tensor("trimask", [128, 128], bf16, kind="ExternalInput")
    outp = nc.dram_tensor("outp", [S, E], bf16, kind="ExternalOutput")

    with tile.TileContext(nc) as tc:
        with (
            tc.tile_pool(name="consts", bufs=1) as consts,
            tc.tile_pool(name="acts", bufs=1) as acts,
            tc.tile_pool(name="slabs", bufs=4) as slabs,
            tc.tile_pool(name="small", bufs=2) as small,
            tc.tile_pool(name="outs", bufs=2) as outs,
            tc.tile_pool(name="ps", bufs=1, space="PSUM") as ps,
        ):
            # ---- constant / weight DMAs (sync queue, deadline order) ----
            bqk_sb = consts.tile([128, 4], f32, tag="bqk")
            nc.sync.dma_start(bqk_sb[:], bqk[:, :])
            vb_sb = consts.tile([128, 260], f32, tag="vb")
            nc.sync.dma_start(vb_sb[:], vb[:, :])
            tri_sb = consts.tile([128, 128], bf16, tag="tri")
            nc.sync.dma_start(tri_sb[:], trimask[:, :])

            # warm up the ACT exp table early (table load ~2.7us)
            warm = consts.tile([128, 4], f32, tag="warm")
            nc.scalar.activation(warm[:], bqk_sb[:], FT.Exp)

            # weights on the (early-idle) gpsimd DMA queue, x-tiles in order
            # on sync, wp on vector — parallel rings, x arrives tile-by-tile
            xt_sb = []
            wqk_sb = []
            wv_sb = []
            for t in range(NET):
                wq = consts.tile([128, 2 * DP], bf16, tag=f"wqk{t}")
                nc.gpsimd.dma_start(wq[:], wqk[t * 128:(t + 1) * 128, :])
                wqk_sb.append(wq)
                wvt = consts.tile([128, 260], bf16, tag=f"wv{t}")
                nc.gpsimd.dma_start(wvt[:], wv[t * 128:(t + 1) * 128, :])
                wv_sb.append(wvt)
                xt = consts.tile([128, S], bf16, tag=f"xt{t}")
                nc.sync.dma_start(xt[:], xT[t * 128:(t + 1) * 128, :])
                xt_sb.append(xt)
            wp_sb = []
            for t in range(2):
                wpt = consts.tile([128, E], bf16, tag=f"wp{t}")
                nc.scalar.dma_start(wpt[:], wp[t * 128:(t + 1) * 128, :])
                wp_sb.append(wpt)

            # ---- persistent activations ----
            v_sb = [acts.tile([128, 260], bf16, tag=f"v{st}", name=f"v{st}")
                    for st in range(NST)]
            qkt_sb = [acts.tile([128, S], bf16, tag=f"qkt{t}", name=f"qkt{t}")
                      for t in range(4)]
            attnT_sb = [acts.tile([128, S], bf16, tag=f"attnT{t}",
                                  name=f"attnT{t}") for t in range(2)]

            # ---- projection pair-group emitters ----
            def qk_pair(tA, scA, tB, scB, tag):
                """two QK projection column groups through one 2-bank psum."""
                p = ps.tile([128, 1024], f32, tag=tag, bufs=PS_BUFS[tag],
                            name=f"qkp{tA}{scA}{tB}{scB}")
                for kt in range(NET):
                    nc.tensor.matmul(
                        p[:, 0:512],
                        (wqk_sb[kt][:, tA * 128:(tA + 1) * 128]),
                        (xt_sb[kt][:, scA * 512:(scA + 1) * 512]),
                        start=(kt == 0), stop=(kt == NET - 1))
                    nc.tensor.matmul(
                        p[:, 512:1024],
                        (wqk_sb[kt][:, tB * 128:(tB + 1) * 128]),
                        (xt_sb[kt][:, scB * 512:(scB + 1) * 512]),
                        start=(kt == 0), stop=(kt == NET - 1))
                nc.vector.tensor_scalar_add(
                    qkt_sb[tA][:, scA * 512:(scA + 1) * 512],
                    p[:, 0:512], bqk_sb[:, tA:tA + 1])
                nc.vector.tensor_scalar_add(
                    qkt_sb[tB][:, scB * 512:(scB + 1) * 512],
                    p[:, 512:1024], bqk_sb[:, tB:tB + 1])

            def v_pair(st0, st1, tag):
                p = ps.tile([128, 1024], f32, tag=tag, bufs=PS_BUFS[tag],
                            name=f"vp{st0}")
                for kt in range(NET):
                    nc.tensor.matmul(
                        p[:, 0:260],
                        (xt_sb[kt][:, st0 * 128:(st0 + 1) * 128]),
                        (wv_sb[kt][:]),
                        start=(kt == 0), stop=(kt == NET - 1))
                    nc.tensor.matmul(
                        p[:, 512:772],
                        (xt_sb[kt][:, st1 * 128:(st1 + 1) * 128]),
                        (wv_sb[kt][:]),
                        start=(kt == 0), stop=(kt == NET - 1))
                nc.vector.tensor_add(v_sb[st0][:], p[:, 0:260], vb_sb[:])
                nc.vector.tensor_add(v_sb[st1][:], p[:, 512:772], vb_sb[:])

            def v_single(st, tag):
                p = ps.tile([128, 512], f32, tag=tag, bufs=PS_BUFS[tag],
                            name=f"vs{st}")
                for kt in range(NET):
                    nc.tensor.matmul(
                        p[:, 0:260],
                        (xt_sb[kt][:, st * 128:(st + 1) * 128]),
                        (wv_sb[kt][:]),
                        start=(kt == 0), stop=(kt == NET - 1))
                nc.vector.tensor_add(v_sb[st][:], p[:, 0:260], vb_sb[:])

            def cproj_pair(st):
                """c_proj for s-tile st, both 512-col output chunks."""
                p = ps.tile([128, 1024], f32, tag="pj", bufs=PS_BUFS["pj"],
                            name=f"cp{st}")
                for nchk in range(2):
                    for i2 in range(2):
                        nc.tensor.matmul(
                            p[:, nchk * 512:(nchk + 1) * 512],
                            (attnT_sb[i2][:, st * 128:(st + 1) * 128]),
                            (wp_sb[i2][:, nchk * 512:(nchk + 1) * 512]),
                            start=(i2 == 0), stop=(i2 == 1))
                ob = outs.tile([128, 1024], bf16, tag="ob")
                nc.vector.tensor_copy(ob[:], p[:])
                nc.sync.dma_start(outp[st * 128:(st + 1) * 128, :], ob[:])

            PS_BUFS = {"sp": 2, "ot": 2, "pj": 1}

            # ---- filler schedule: (qc, hp, round) -> emitters ----
            fillers = {}

            def place(key, fn):
                fillers.setdefault(key, []).append(fn)

            # pre-attention wave (uses sp/ot slots before attention claims them)
            pre = [
                lambda: qk_pair(0, 0, 2, 0, "sp"),
                lambda: v_pair(0, 1, "sp"),
                lambda: v_single(2, "ot"),
                lambda: v_single(3, "ot"),
            ]
            place((0, 0, 0), lambda: qk_pair(1, 0, 3, 0, "pj"))
            place((0, 0, 1), lambda: qk_pair(0, 1, 1, 1, "pj"))   # QQ sc1
            place((0, 1, 0), lambda: v_pair(4, 5, "pj"))
            place((0, 1, 1), lambda: v_pair(6, 7, "pj"))
            place((1, 0, 0), lambda: qk_pair(2, 1, 3, 1, "pj"))   # KK sc1
            place((1, 0, 1), lambda: qk_pair(0, 2, 1, 2, "pj"))   # QQ sc2
            place((1, 0, 2), lambda: v_pair(8, 9, "pj"))
            place((1, 1, 0), lambda: v_pair(10, 11, "pj"))
            place((1, 1, 1), lambda: cproj_pair(0))
            place((1, 1, 2), lambda: cproj_pair(1))
            place((1, 1, 3), lambda: cproj_pair(2))
            place((2, 0, 0), lambda: cproj_pair(3))
            place((2, 0, 1), lambda: qk_pair(2, 2, 3, 2, "pj"))   # KK sc2
            place((2, 0, 2), lambda: qk_pair(0, 3, 1, 3, "pj"))   # QQ sc3
            place((2, 0, 4), lambda: v_pair(12, 13, "pj"))
            place((2, 1, 0), lambda: v_pair(14, 15, "pj"))
            place((2, 1, 2), lambda: cproj_pair(4))
            place((2, 1, 3), lambda: cproj_pair(5))
            place((2, 1, 4), lambda: cproj_pair(6))
            place((3, 0, 0), lambda: cproj_pair(7))
            place((3, 0, 1), lambda: qk_pair(2, 3, 3, 3, "pj"))   # KK sc3
            place((3, 0, 3), lambda: cproj_pair(8))
            place((3, 0, 5), lambda: cproj_pair(9))
            place((3, 1, 1), lambda: cproj_pair(10))
            place((3, 1, 3), lambda: cproj_pair(11))
            # qc3's own c_proj (st 12-15) must follow qc3's normalize -> tail.
            # Single-bank groups round-robin freed psum slots; casts alternate
            # DVE/ACT so the tail pipelines at PE rate.
            def cproj_single(st, nchk, tag, on_act):
                p = ps.tile([128, 512], f32, tag=tag, bufs=PS_BUFS[tag],
                            name=f"cs{st}_{nchk}")
                for i2 in range(2):
                    nc.tensor.matmul(
                        p[:],
                        (attnT_sb[i2][:, st * 128:(st + 1) * 128]),
                        (wp_sb[i2][:, nchk * 512:(nchk + 1) * 512]),
                        start=(i2 == 0), stop=(i2 == 1))
                ob = outs.tile([128, 512], bf16, tag="obs", bufs=3)
                if on_act:
                    nc.scalar.copy(ob[:], p[:])
                else:
                    nc.vector.tensor_copy(ob[:], p[:])
                nc.sync.dma_start(
                    outp[st * 128:(st + 1) * 128,
                         nchk * 512:(nchk + 1) * 512], ob[:])

            def tail_emit():
                k = 0
                for st in range(12, 16):
                    for nchk in range(2):
                        cproj_single(st, nchk, ("ot", "sp")[k % 2], k % 2 == 0)
                        k += 1
            tail = [tail_emit]

            for fn in pre:
                fn()

            # ---- attention ----
            for qc in range(NQC):
                nkt = 4 * qc + 4
                for hp in range(2):
                    tq, tk = hp, 2 + hp
                    ots = [ps.tile([65, 512], f32, tag="ot",
                                   bufs=PS_BUFS["ot"],
                                   name=f"ot{qc}_{hp}_{i}") for i in range(2)]
                    for r in range(nkt // 2):
                        for fn in fillers.pop((qc, hp, r), []):
                            fn()
                        slabs_r = []
                        for kt in (2 * r, 2 * r + 1):
                            di = kt - 4 * qc
                            o = max(di, 0) * 128
                            sp = ps.tile([128, 1024], f32, tag="sp",
                                         bufs=PS_BUFS["sp"], name="sp")
                            for i in range(2):
                                nc.tensor.matmul(
                                    sp[:, 512 * i + o:512 * i + 512],
                                    (qkt_sb[tk][64 * i:64 * i + 64,
                                                kt * 128:(kt + 1) * 128]),
                                    (qkt_sb[tq][64 * i:64 * i + 64,
                                                qc * 512 + o:(qc + 1) * 512]),
                                    start=True, stop=True)
                            slab = slabs.tile([128, 1024], bf16, tag="slab")
                            if o == 0:
                                nc.scalar.activation(slab[:], sp[:], FT.Exp)
                            else:
                                s3 = slab.rearrange(
                                    "p (h n) -> p h n", h=2)[:, :, o:512]
                                p3 = sp.rearrange(
                                    "p (h n) -> p h n", h=2)[:, :, o:512]
                                nc.scalar.activation(s3, p3, FT.Exp)
                            if di >= 0:
                                for i in range(2):
                                    nc.vector.tensor_mul(
                                        slab[:, 512 * i + o:512 * i + o + 128],
                                        slab[:, 512 * i + o:512 * i + o + 128],
                                        tri_sb[:])
                            slabs_r.append((kt, o, slab))
                        for kt, o, slab in slabs_r:
                            for i in range(2):
                                h = 2 * hp + i
                                nc.tensor.matmul(
                                    ots[i][:, o:512],
                                    (v_sb[kt][:, 65 * h:65 * h + 65]),
                                    (slab[:, 512 * i + o:512 * i + 512]),
                                    start=(kt == 0), stop=(kt == nkt - 1))
                    # normalize: rows 0..63 * (1/Z), Z = row 64
                    for i in range(2):
                        h = 2 * hp + i
                        zrow = small.tile([1, 512], f32, tag="zrow")
                        nc.vector.tensor_copy(zrow[:], ots[i][64:65, :])
                        rz = small.tile([1, 512], f32, tag="rz")
                        nc.vector.reciprocal_approx_fast(rz[:], zrow[:])
                        sbb = small.tile([64, 512], f32, tag="sbb")
                        nc.gpsimd.partition_broadcast(sbb[:], rz[0:1, :])
                        nc.vector.tensor_mul(
                            attnT_sb[hp][64 * i:64 * i + 64,
                                         qc * 512:(qc + 1) * 512],
                            ots[i][0:64, :], sbb[:])
            for key in sorted(fillers):
                for fn in fillers[key]:
                    fn()
            for fn in tail:
                fn()

    nc.finalize()
    return nc


def _prep_inputs(hidden_states, w_attn, b_attn, w_proj, b_proj):
    hs = np.asarray(hidden_states, np.float32)
    wa = np.asarray(w_attn, np.float32)
    ba = np.asarray(b_attn, np.float32)
    wpj = np.asarray(w_proj, np.float32)

    import ml_dtypes
    bfl = ml_dtypes.bfloat16
    xTs = [np.ascontiguousarray(hs[b].T.astype(bfl)) for b in range(B)]
    triu = (np.arange(128)[:, None] <= np.arange(128)[None, :])
    trimask = triu.astype(bfl)

    in_maps = []
    for c in range(8):
        b, g = c // 4, c % 4
        q0 = DP * g
        k0 = E + DP * g
        v0 = 2 * E + DP * g
        wqk = np.concatenate(
            [wa[:, q0:q0 + DP] * 0.125, wa[:, k0:k0 + DP]], axis=1).astype(bfl)
        bqk = np.zeros((128, 4), np.float32)
        bqk[:, 0] = ba[q0:q0 + 128] * 0.125
        bqk[:, 1] = ba[q0 + 128:q0 + 256] * 0.125
        bqk[:, 2] = ba[k0:k0 + 128]
        bqk[:, 3] = ba[k0 + 128:k0 + 256]
        wv = np.zeros((E, 260), bfl)
        vb = np.zeros((128, 260), np.float32)
        for h in range(HPC):
            wv[:, 65 * h:65 * h + 64] = wa[:, v0 + 64 * h:v0 + 64 * h + 64].astype(bfl)
            vb[:, 65 * h:65 * h + 64] = ba[v0 + 64 * h:v0 + 64 * h + 64]
            vb[:, 65 * h + 64] = 1.0
        wp = np.ascontiguousarray(wpj[DP * g:DP * (g + 1), :].astype(bfl))
        in_maps.append({
            "xT": xTs[b],
            "wqk": np.ascontiguousarray(wqk),
            "bqk": bqk,
            "wv": wv,
            "vb": vb,
            "wp": wp,
            "trimask": trimask,
        })
    return in_maps


def run(trace=False, **inputs):
    if "nc" not in _CACHED:
        _CACHED["nc"] = build_nc()
    nc = _CACHED["nc"]
    in_maps = _prep_inputs(**inputs)
    res = run_bass_kernel_spmd(nc, in_maps, list(range(8)), trace=trace)
    b_proj = np.asarray(inputs["b_proj"], np.float32)
    out = np.empty((B, S, E), np.float32)
    for b in range(B):
        acc = res.results[4 * b]["outp"].astype(np.float32)
        for g in range(1, 4):
            acc = acc + res.results[4 * b + g]["outp"].astype(np.float32)
        out[b] = acc + b_proj
    return out, res


def kernel(**inputs):
    out, _ = run(trace=False, **inputs)
    return out
